# revision 2
# baseline (speedup 1.0000x reference)
"""Causal self-attention (B=4, T=2048, C=1024, H=16) on 8 Trainium2 NeuronCores.

Sharding: core = (batch b, head-group hg) with b in 0..3, hg in {0,1}.
Each core computes qkv projection, causal attention and a partial output
projection for its 8 heads of its batch; the host sums the two head-group
partials per batch (the TP unshard step).

All matmuls run in float32r (full PE rate at moving-dim >= 256, ~2^-12
rounding). Scores are computed transposed (scoresT[k, q]) so the PV matmul
directly yields transposed head outputs; a ones-column appended to V yields
the softmax denominators from the same matmul. Softmax skips the max
subtraction (logits are ~N(0,1), |logit| < 10 for this problem's scale).
"""

import numpy as np

B, T, C = 4, 2048, 1024
H, DH = 16, 64
HG = 2                # head groups (tensor parallel)
HPG = H // HG         # heads per group
GC = HPG * DH         # 512 channels per group
NCORES = 8
QB = 256              # query block (matmul moving dim)
KB = 128              # key chunk
CK = C // 128         # contraction chunks over C
NT = T // 512         # 512-wide column chunks over T
TM = T // KB          # key chunks over T
MQK = 2 * GC // 128   # output row chunks for q|k projection
MO = C // 128         # out-proj output chunks
KO = GC // 128        # out-proj contraction chunks

_CACHE = {}


def _build_nc():
    import concourse.mybir as mybir
    import concourse.tile as tile
    from concourse import bacc

    F32 = mybir.dt.float32
    F32R = mybir.dt.float32r
    AF = mybir.ActivationFunctionType

    nc = bacc.Bacc(
        "TRN2", target_bir_lowering=False, debug=False, num_devices=NCORES
    )

    xt_d = nc.dram_tensor("xt", [C, T], F32, kind="ExternalInput")
    wqk_d = nc.dram_tensor("wqk", [C, 2 * GC], F32, kind="ExternalInput")
    wv_d = nc.dram_tensor("wv", [C, GC], F32, kind="ExternalInput")
    wo_d = nc.dram_tensor("wo", [GC, C], F32, kind="ExternalInput")
    bqk_d = nc.dram_tensor("bqk", [128, MQK], F32, kind="ExternalInput")
    bv_d = nc.dram_tensor("bv", [1, GC], F32, kind="ExternalInput")
    bo_d = nc.dram_tensor("bo", [128, MO], F32, kind="ExternalInput")
    mask_d = nc.dram_tensor("mask", [128, 2 * QB], F32, kind="ExternalInput")
    out_d = nc.dram_tensor("outp", [C, T], F32, kind="ExternalOutput")

    with tile.TileContext(nc) as tc:
        with tc.tile_pool(name="persist", bufs=1) as pp:
            qk_sb = [
                pp.tile([128, T], F32R, name=f"qk{m}", tag=f"qk{m}")
                for m in range(MQK)
            ]
            v_sb = [
                pp.tile([128, HPG, DH + 1], F32R, name=f"v{t}", tag=f"v{t}")
                for t in range(TM)
            ]
            bqk_sb = pp.tile([128, MQK], F32, name="bqk_sb")
            bo_sb = pp.tile([128, MO], F32, name="bo_sb")
            bvr_sb = pp.tile([1, GC], F32, name="bvr_sb")
            bvb_sb = pp.tile([128, GC], F32, name="bvb_sb")
            mask_sb = pp.tile([128, 2 * QB], F32, name="mask_sb")
            nc.sync.dma_start(bqk_sb[:], bqk_d[:])
            nc.sync.dma_start(bo_sb[:], bo_d[:])
            nc.sync.dma_start(bvr_sb[:], bv_d[:])
            nc.sync.dma_start(mask_sb[:], mask_d[:])
            nc.gpsimd.partition_broadcast(bvb_sb[:], bvr_sb[:])

            # ---- phase 1: qkv projection (qT, kT transposed; v straight) ----
            with (
                tc.tile_pool(name="p1x", bufs=2) as p1x,
                tc.tile_pool(name="p1w", bufs=1) as p1w,
                tc.tile_pool(name="ps1", bufs=3, space="PSUM") as ps1,
            ):
                wqk_sb = [
                    p1w.tile([128, CK, 128], F32R, name=f"wqk{m}", tag=f"wqk{m}")
                    for m in range(MQK)
                ]
                for m in range(MQK):
                    nc.sync.dma_start(
                        wqk_sb[m][:],
                        wqk_d[:, m * 128 : (m + 1) * 128]
                        .rearrange("(c p) m -> p c m", p=128)
                        .bitcast(F32R),
                    )
                wv_sb = p1w.tile([128, CK, GC], F32R, name="wv_sb")
                nc.sync.dma_start(
                    wv_sb[:],
                    wv_d.ap().rearrange("(c p) v -> p c v", p=128).bitcast(F32R),
                )
                for n in range(NT):
                    xt_n = p1x.tile([128, CK, 512], F32R, name="xt_n", tag="xt_n")
                    nc.sync.dma_start(
                        xt_n[:],
                        xt_d[:, n * 512 : (n + 1) * 512]
                        .rearrange("(c p) t -> p c t", p=128)
                        .bitcast(F32R),
                    )
                    for m in range(MQK):
                        ps = ps1.tile([128, 512], F32, name="ps_qk", tag="ps1")
                        for c in range(CK):
                            nc.tensor.matmul(
                                ps[:],
                                wqk_sb[m][:, c, :],
                                xt_n[:, c, :],
                                start=(c == 0),
                                stop=(c == CK - 1),
                            )
                        nc.vector.tensor_scalar_add(
                            qk_sb[m][:, n * 512 : (n + 1) * 512],
                            ps[:],
                            bqk_sb[:, m : m + 1],
                        )
                    for t in range(4):
                        tm = n * 4 + t
                        ps = ps1.tile([128, GC], F32, name="ps_v", tag="ps1")
                        for c in range(CK):
                            nc.tensor.matmul(
                                ps[:],
                                xt_n[:, c, t * 128 : (t + 1) * 128],
                                wv_sb[:, c, :],
                                start=(c == 0),
                                stop=(c == CK - 1),
                            )
                        nc.vector.tensor_tensor(
                            v_sb[tm][:, :, 0:DH],
                            ps[:].rearrange("p (h d) -> p h d", h=HPG),
                            bvb_sb[:].rearrange("p (h d) -> p h d", h=HPG),
                            mybir.AluOpType.add,
                        )
                        nc.gpsimd.memset(
                            v_sb[tm][:, :, DH : DH + 1].bitcast(F32), 1.0
                        )

            # ---- phase 2: attention, phase 3: output projection ----
            with (
                tc.tile_pool(name="p23", bufs=1) as p23,
                tc.tile_pool(name="spool", bufs=4) as spool,
                tc.tile_pool(name="rpool", bufs=3) as rpool,
                tc.tile_pool(name="pss", bufs=3, space="PSUM") as pss,
                tc.tile_pool(name="pso", bufs=2, space="PSUM") as pso,
                tc.tile_pool(name="ps3", bufs=2, space="PSUM") as ps3,
                tc.tile_pool(name="opool", bufs=3) as opool,
            ):
                ho_sb = [
                    p23.tile([128, T], F32R, name=f"ho{i}", tag=f"ho{i}")
                    for i in range(KO)
                ]
                wo_sb = [
                    p23.tile([128, MO, 128], F32R, name=f"wo{c}", tag=f"wo{c}")
                    for c in range(KO)
                ]
                for c2 in range(KO):
                    nc.sync.dma_start(
                        wo_sb[c2][:],
                        wo_d[c2 * 128 : (c2 + 1) * 128, :]
                        .rearrange("p (m i) -> p m i", i=128)
                        .bitcast(F32R),
                    )
                for h in range(HPG):
                    ch = h // 2
                    off = (h % 2) * 64
                    for qb in range(T // QB):
                        qo = qb * QB
                        nk = 2 * qb + 2
                        po = pso.tile([DH + 1, QB], F32, name="po", tag="po")
                        for kc in range(nk):
                            ps = pss.tile([128, QB], F32, name="ps_s", tag="pss")
                            nc.tensor.matmul(
                                ps[:],
                                qk_sb[MQK // 2 + ch][
                                    off : off + 64, kc * KB : (kc + 1) * KB
                                ],
                                qk_sb[ch][off : off + 64, qo : qo + QB],
                                start=True,
                                stop=True,
                            )
                            s = spool.tile([128, QB], F32R, name="s_sb", tag="s")
                            nc.scalar.activation(s[:], ps[:], AF.Exp, scale=0.125)
                            di = kc - (nk - 2)
                            if di >= 0:
                                nc.vector.tensor_mul(
                                    s[:], s[:], mask_sb[:, di * QB : (di + 1) * QB]
                                )
                            nc.tensor.matmul(
                                po[:],
                                v_sb[kc][:, h, :],
                                s[:],
                                start=(kc == 0),
                                stop=(kc == nk - 1),
                            )
                        r = rpool.tile([1, QB], F32, name="r_sb", tag="r")
                        nc.vector.reciprocal(r[:], po[DH : DH + 1, :])
                        rb = rpool.tile([64, QB], F32, name="rb_sb", tag="rb")
                        nc.gpsimd.partition_broadcast(rb[:], r[:])
                        nc.vector.tensor_mul(
                            ho_sb[ch][off : off + 64, qo : qo + QB],
                            po[0:DH, :],
                            rb[:],
                        )
                for m in range(MO):
                    for n2 in range(NT):
                        ps = ps3.tile([128, 512], F32, name="ps_o", tag="ps3")
                        for c2 in range(KO):
                            nc.tensor.matmul(
                                ps[:],
                                wo_sb[c2][:, m, :],
                                ho_sb[c2][:, n2 * 512 : (n2 + 1) * 512],
                                start=(c2 == 0),
                                stop=(c2 == KO - 1),
                            )
                        ot = opool.tile([128, 512], F32, name="ot", tag="ot")
                        nc.vector.tensor_scalar_add(
                            ot[:], ps[:], bo_sb[:, m : m + 1]
                        )
                        nc.sync.dma_start(
                            out_d[m * 128 : (m + 1) * 128, n2 * 512 : (n2 + 1) * 512],
                            ot[:],
                        )

    nc.compile()
    return nc


def _get_nc():
    if "nc" not in _CACHE:
        _CACHE["nc"] = _build_nc()
    return _CACHE["nc"]


def _make_in_maps(x, w_qkv, b_qkv, w_out, b_out):
    x = np.ascontiguousarray(np.asarray(x, dtype=np.float32))
    w_qkv = np.asarray(w_qkv, dtype=np.float32)
    b_qkv = np.asarray(b_qkv, dtype=np.float32)
    w_out = np.asarray(w_out, dtype=np.float32)
    b_out = np.asarray(b_out, dtype=np.float32)

    j = np.arange(QB)[None, :]
    k = np.arange(128)[:, None]
    m0 = (k <= j).astype(np.float32)
    m1 = (k + 128 <= j).astype(np.float32)
    mask = np.ascontiguousarray(np.concatenate([m0, m1], axis=1))

    per_hg = {}
    for hg in range(HG):
        qs = slice(hg * GC, (hg + 1) * GC)
        ks = slice(C + hg * GC, C + (hg + 1) * GC)
        vs = slice(2 * C + hg * GC, 2 * C + (hg + 1) * GC)
        wqk_t = np.ascontiguousarray(
            np.concatenate([w_qkv[qs], w_qkv[ks]], axis=0).T
        )
        wv_t = np.ascontiguousarray(w_qkv[vs].T)
        wo_t = np.ascontiguousarray(w_out[:, hg * GC : (hg + 1) * GC].T)
        bqk = np.ascontiguousarray(
            np.concatenate([b_qkv[qs], b_qkv[ks]]).reshape(MQK, 128).T
        )
        bv = np.ascontiguousarray(b_qkv[vs].reshape(1, GC))
        bo_vec = b_out if hg == 0 else np.zeros_like(b_out)
        bo = np.ascontiguousarray(bo_vec.reshape(MO, 128).T)
        per_hg[hg] = (wqk_t, wv_t, wo_t, bqk, bv, bo)

    in_maps = []
    for cid in range(NCORES):
        b, hg = cid // HG, cid % HG
        wqk_t, wv_t, wo_t, bqk, bv, bo = per_hg[hg]
        in_maps.append(
            {
                "xt": np.ascontiguousarray(x[b].T),
                "wqk": wqk_t,
                "wv": wv_t,
                "wo": wo_t,
                "bqk": bqk,
                "bv": bv,
                "bo": bo,
                "mask": mask,
            }
        )
    return in_maps


def _run(in_maps, **kwargs):
    from concourse.bass_utils import run_bass_kernel_spmd

    nc = _get_nc()
    return run_bass_kernel_spmd(nc, in_maps, core_ids=list(range(NCORES)), **kwargs)


def kernel(x, w_qkv, b_qkv, w_out, b_out):
    in_maps = _make_in_maps(x, w_qkv, b_qkv, w_out, b_out)
    res = _run(in_maps)
    out = np.empty((B, T, C), dtype=np.float32)
    for b in range(B):
        acc = res.results[b * HG]["outp"] + res.results[b * HG + 1]["outp"]
        out[b] = acc.T
    return out


if __name__ == "__main__":
    rng = np.random.default_rng(0)
    x = rng.standard_normal((B, T, C), dtype=np.float32)
    w_qkv = rng.standard_normal((3 * C, C), dtype=np.float32) / np.sqrt(C)
    b_qkv = np.zeros(3 * C, dtype=np.float32)
    w_out = rng.standard_normal((C, C), dtype=np.float32) / np.sqrt(C)
    b_out = np.zeros(C, dtype=np.float32)
    out = kernel(x, w_qkv, b_qkv, w_out, b_out)
    print("out", out.shape, out.dtype, np.abs(out).max())


# revision 4
# speedup vs baseline: 1.0715x; 1.0715x over previous
"""Causal self-attention (B=4, T=2048, C=1024, H=16) on 8 Trainium2 NeuronCores.

Sharding: core = (batch b, head-group hg) with b in 0..3, hg in {0,1}.
Each core computes qkv projection, causal attention and a partial output
projection for its 8 heads of its batch; the host sums the two head-group
partials per batch (the TP unshard step).

All matmuls run in float32r (~2^-12 rounding; full PE rate needs moving
dim 512). Scores are computed transposed (scoresT[k, q]) so the PV matmul
directly yields transposed head outputs; a ones-column appended to V yields
the softmax denominators from the same matmul. The causal mask is folded
into the QK PSUM accumulation as an identity-matmul adding -60 to masked
logits, so exp() zeroes them with no vector-engine masking pass. Softmax
skips the max subtraction (logits are ~N(0,1), |logit| < 10 at this scale).
"""

import numpy as np

B, T, C = 4, 2048, 1024
H, DH = 16, 64
HG = 2                # head groups (tensor parallel)
HPG = H // HG         # heads per group
GC = HPG * DH         # 512 channels per group
NCORES = 8
QB = 512              # query block (matmul moving dim)
KB = 128              # key chunk
CK = C // 128         # contraction chunks over C
NT = T // 512         # 512-wide column chunks over T
TM = T // KB          # key chunks over T
MQK = 2 * GC // 128   # output row chunks for q|k projection
MO = C // 128         # out-proj output chunks
KO = GC // 128        # out-proj contraction chunks
NQB = T // QB         # query blocks
MASK_NEG = -480.0  # pre-scaled: exp scale=0.125 turns this into -60 on the logit

_CACHE = {}


def _build_nc():
    import concourse.mybir as mybir
    import concourse.tile as tile
    from concourse import bacc

    F32 = mybir.dt.float32
    F32R = mybir.dt.float32r
    AF = mybir.ActivationFunctionType

    nc = bacc.Bacc(
        "TRN2", target_bir_lowering=False, debug=False, num_devices=NCORES
    )

    xt_d = nc.dram_tensor("xt", [C, T], F32, kind="ExternalInput")
    wqk_d = nc.dram_tensor("wqk", [C, 2 * GC], F32, kind="ExternalInput")
    wv_d = nc.dram_tensor("wv", [C, GC], F32, kind="ExternalInput")
    wo_d = nc.dram_tensor("wo", [GC, C], F32, kind="ExternalInput")
    bqk_d = nc.dram_tensor("bqk", [128, MQK], F32, kind="ExternalInput")
    bv_d = nc.dram_tensor("bv", [1, GC], F32, kind="ExternalInput")
    bo_d = nc.dram_tensor("bo", [128, MO], F32, kind="ExternalInput")
    mask_d = nc.dram_tensor("mask", [128, 4 * QB], F32, kind="ExternalInput")
    idn_d = nc.dram_tensor("idn", [128, 128], F32, kind="ExternalInput")
    out_d = nc.dram_tensor("outp", [C, T], F32, kind="ExternalOutput")

    with tile.TileContext(nc) as tc:
        with tc.tile_pool(name="persist", bufs=1) as pp:
            qk_sb = [
                pp.tile([128, T], F32R, name=f"qk{m}", tag=f"qk{m}")
                for m in range(MQK)
            ]
            v_sb = [
                pp.tile([128, HPG, DH + 1], F32R, name=f"v{t}", tag=f"v{t}")
                for t in range(TM)
            ]
            bqk_sb = pp.tile([128, MQK], F32, name="bqk_sb")
            bo_sb = pp.tile([128, MO], F32, name="bo_sb")
            bvr_sb = pp.tile([1, GC], F32, name="bvr_sb")
            bvb_sb = pp.tile([128, GC], F32, name="bvb_sb")
            mask_sb = pp.tile([128, 4, QB], F32R, name="mask_sb")
            idn_sb = pp.tile([128, 128], F32R, name="idn_sb")
            nc.sync.dma_start(bqk_sb[:], bqk_d[:])
            nc.sync.dma_start(bo_sb[:], bo_d[:])
            nc.sync.dma_start(bvr_sb[:], bv_d[:])
            nc.sync.dma_start(
                mask_sb[:],
                mask_d.ap().rearrange("p (d q) -> p d q", d=4).bitcast(F32R),
            )
            nc.sync.dma_start(idn_sb[:], idn_d[:].bitcast(F32R))
            nc.gpsimd.partition_broadcast(bvb_sb[:], bvr_sb[:])

            # ---- phase 1: qkv projection (qT, kT transposed; v straight) ----
            with (
                tc.tile_pool(name="p1x", bufs=2) as p1x,
                tc.tile_pool(name="p1w", bufs=1) as p1w,
                tc.tile_pool(name="ps1", bufs=3, space="PSUM") as ps1,
            ):
                wqk_sb = [
                    p1w.tile([128, CK, 128], F32R, name=f"wqk{m}", tag=f"wqk{m}")
                    for m in range(MQK)
                ]
                for m in range(MQK):
                    nc.sync.dma_start(
                        wqk_sb[m][:],
                        wqk_d[:, m * 128 : (m + 1) * 128]
                        .rearrange("(c p) m -> p c m", p=128)
                        .bitcast(F32R),
                    )
                wv_sb = p1w.tile([128, CK, GC], F32R, name="wv_sb")
                nc.sync.dma_start(
                    wv_sb[:],
                    wv_d.ap().rearrange("(c p) v -> p c v", p=128).bitcast(F32R),
                )
                for n in range(NT):
                    xt_n = p1x.tile([128, CK, 512], F32R, name="xt_n", tag="xt_n")
                    nc.sync.dma_start(
                        xt_n[:],
                        xt_d[:, n * 512 : (n + 1) * 512]
                        .rearrange("(c p) t -> p c t", p=128)
                        .bitcast(F32R),
                    )
                    for m in range(MQK):
                        ps = ps1.tile([128, 512], F32, name="ps_qk", tag="ps1")
                        for c in range(CK):
                            nc.tensor.matmul(
                                ps[:],
                                wqk_sb[m][:, c, :],
                                xt_n[:, c, :],
                                start=(c == 0),
                                stop=(c == CK - 1),
                            )
                        nc.vector.tensor_scalar_add(
                            qk_sb[m][:, n * 512 : (n + 1) * 512],
                            ps[:],
                            bqk_sb[:, m : m + 1],
                        )
                    for t in range(4):
                        tm = n * 4 + t
                        ps = ps1.tile([128, GC], F32, name="ps_v", tag="ps1")
                        for c in range(CK):
                            nc.tensor.matmul(
                                ps[:],
                                xt_n[:, c, t * 128 : (t + 1) * 128],
                                wv_sb[:, c, :],
                                start=(c == 0),
                                stop=(c == CK - 1),
                            )
                        nc.vector.tensor_tensor(
                            v_sb[tm][:, :, 0:DH],
                            ps[:].rearrange("p (h d) -> p h d", h=HPG),
                            bvb_sb[:].rearrange("p (h d) -> p h d", h=HPG),
                            mybir.AluOpType.add,
                        )
                        nc.gpsimd.memset(
                            v_sb[tm][:, :, DH : DH + 1].bitcast(F32), 1.0
                        )

            # ---- phase 2: attention, phase 3: output projection ----
            with (
                tc.tile_pool(name="p23", bufs=1) as p23,
                tc.tile_pool(name="spool", bufs=3) as spool,
                tc.tile_pool(name="rpool", bufs=3) as rpool,
                tc.tile_pool(name="pss", bufs=2, space="PSUM") as pss,
                tc.tile_pool(name="pso", bufs=2, space="PSUM") as pso,
                tc.tile_pool(name="ps3", bufs=2, space="PSUM") as ps3,
                tc.tile_pool(name="opool", bufs=3) as opool,
            ):
                ho_sb = [
                    p23.tile([128, T], F32R, name=f"ho{i}", tag=f"ho{i}")
                    for i in range(KO)
                ]
                wo_sb = [
                    p23.tile([128, MO, 128], F32R, name=f"wo{c}", tag=f"wo{c}")
                    for c in range(KO)
                ]
                for c2 in range(KO):
                    nc.sync.dma_start(
                        wo_sb[c2][:],
                        wo_d[c2 * 128 : (c2 + 1) * 128, :]
                        .rearrange("p (m i) -> p m i", i=128)
                        .bitcast(F32R),
                    )
                for h in range(HPG):
                    ch = h // 2
                    off = (h % 2) * 64
                    for qb in range(NQB):
                        qo = qb * QB
                        nk = 4 * qb + 4
                        po = pso.tile([DH + 1, QB], F32, name="po", tag="po")
                        for kc0 in range(0, nk, 2):
                            ps2 = pss.tile([128, 2, QB], F32, name="ps_s", tag="pss")
                            for j in range(2):
                                kc = kc0 + j
                                di = kc - (nk - 4)
                                nc.tensor.matmul(
                                    ps2[:, j, :],
                                    qk_sb[MQK // 2 + ch][
                                        off : off + 64, kc * KB : (kc + 1) * KB
                                    ],
                                    qk_sb[ch][off : off + 64, qo : qo + QB],
                                    start=True,
                                    stop=(di < 0),
                                )
                                if di >= 0:
                                    nc.tensor.matmul(
                                        ps2[:, j, :],
                                        idn_sb[:],
                                        mask_sb[:, di, :],
                                        start=False,
                                        stop=True,
                                    )
                            s2 = spool.tile([128, 2, QB], F32R, name="s_sb", tag="s")
                            nc.scalar.activation(s2[:], ps2[:], AF.Exp, scale=0.125)
                            for j in range(2):
                                kc = kc0 + j
                                nc.tensor.matmul(
                                    po[:],
                                    v_sb[kc][:, h, :],
                                    s2[:, j, :],
                                    start=(kc == 0),
                                    stop=(kc == nk - 1),
                                )
                        dsb = rpool.tile([1, QB], F32, name="d_sb", tag="d")
                        nc.vector.tensor_copy(dsb[:], po[DH : DH + 1, :])
                        r = rpool.tile([1, QB], F32, name="r_sb", tag="r")
                        # approx_fast is wrong when reading PSUM; feed it SBUF
                        nc.vector.reciprocal_approx_fast(r[:], dsb[:])
                        rb = rpool.tile([64, QB], F32, name="rb_sb", tag="rb")
                        nc.gpsimd.partition_broadcast(rb[:], r[:])
                        nc.vector.tensor_mul(
                            ho_sb[ch][off : off + 64, qo : qo + QB],
                            po[0:DH, :],
                            rb[:],
                        )
                for m in range(MO):
                    for n2 in range(NT):
                        ps = ps3.tile([128, 512], F32, name="ps_o", tag="ps3")
                        for c2 in range(KO):
                            nc.tensor.matmul(
                                ps[:],
                                wo_sb[c2][:, m, :],
                                ho_sb[c2][:, n2 * 512 : (n2 + 1) * 512],
                                start=(c2 == 0),
                                stop=(c2 == KO - 1),
                            )
                        ot = opool.tile([128, 512], F32, name="ot", tag="ot")
                        nc.vector.tensor_scalar_add(
                            ot[:], ps[:], bo_sb[:, m : m + 1]
                        )
                        nc.sync.dma_start(
                            out_d[m * 128 : (m + 1) * 128, n2 * 512 : (n2 + 1) * 512],
                            ot[:],
                        )

    nc.compile()
    return nc


def _get_nc():
    if "nc" not in _CACHE:
        _CACHE["nc"] = _build_nc()
    return _CACHE["nc"]


def _make_in_maps(x, w_qkv, b_qkv, w_out, b_out):
    x = np.ascontiguousarray(np.asarray(x, dtype=np.float32))
    w_qkv = np.asarray(w_qkv, dtype=np.float32)
    b_qkv = np.asarray(b_qkv, dtype=np.float32)
    w_out = np.asarray(w_out, dtype=np.float32)
    b_out = np.asarray(b_out, dtype=np.float32)

    j = np.arange(QB)[None, :]
    k = np.arange(128)[:, None]
    mask = np.concatenate(
        [
            np.where(di * 128 + k <= j, 0.0, MASK_NEG).astype(np.float32)
            for di in range(4)
        ],
        axis=1,
    )
    mask = np.ascontiguousarray(mask)
    idn = np.eye(128, dtype=np.float32)

    per_hg = {}
    for hg in range(HG):
        qs = slice(hg * GC, (hg + 1) * GC)
        ks = slice(C + hg * GC, C + (hg + 1) * GC)
        vs = slice(2 * C + hg * GC, 2 * C + (hg + 1) * GC)
        wqk_t = np.ascontiguousarray(
            np.concatenate([w_qkv[qs], w_qkv[ks]], axis=0).T
        )
        wv_t = np.ascontiguousarray(w_qkv[vs].T)
        wo_t = np.ascontiguousarray(w_out[:, hg * GC : (hg + 1) * GC].T)
        bqk = np.ascontiguousarray(
            np.concatenate([b_qkv[qs], b_qkv[ks]]).reshape(MQK, 128).T
        )
        bv = np.ascontiguousarray(b_qkv[vs].reshape(1, GC))
        bo_vec = b_out if hg == 0 else np.zeros_like(b_out)
        bo = np.ascontiguousarray(bo_vec.reshape(MO, 128).T)
        per_hg[hg] = (wqk_t, wv_t, wo_t, bqk, bv, bo)

    in_maps = []
    for cid in range(NCORES):
        b, hg = cid // HG, cid % HG
        wqk_t, wv_t, wo_t, bqk, bv, bo = per_hg[hg]
        in_maps.append(
            {
                "xt": np.ascontiguousarray(x[b].T),
                "wqk": wqk_t,
                "wv": wv_t,
                "wo": wo_t,
                "bqk": bqk,
                "bv": bv,
                "bo": bo,
                "mask": mask,
                "idn": idn,
            }
        )
    return in_maps


def _run(in_maps, **kwargs):
    from concourse.bass_utils import run_bass_kernel_spmd

    nc = _get_nc()
    return run_bass_kernel_spmd(nc, in_maps, core_ids=list(range(NCORES)), **kwargs)


def kernel(x, w_qkv, b_qkv, w_out, b_out):
    in_maps = _make_in_maps(x, w_qkv, b_qkv, w_out, b_out)
    res = _run(in_maps)
    out = np.empty((B, T, C), dtype=np.float32)
    for b in range(B):
        acc = res.results[b * HG]["outp"] + res.results[b * HG + 1]["outp"]
        out[b] = acc.T
    return out


if __name__ == "__main__":
    rng = np.random.default_rng(0)
    x = rng.standard_normal((B, T, C), dtype=np.float32)
    w_qkv = rng.standard_normal((3 * C, C), dtype=np.float32) / np.sqrt(C)
    b_qkv = np.zeros(3 * C, dtype=np.float32)
    w_out = rng.standard_normal((C, C), dtype=np.float32) / np.sqrt(C)
    b_out = np.zeros(C, dtype=np.float32)
    out = kernel(x, w_qkv, b_qkv, w_out, b_out)
    print("out", out.shape, out.dtype, np.abs(out).max())
